# revision 1
# baseline (speedup 1.0000x reference)
"""Trainium2 Bass kernel for nn_MultiHeadAttention_67044439491211.

Mathematical note: the reference einsum 'bqkh,bvha->bqha' sums k and v
independently, so attn = (sum_k softmax(...)) * (sum_v v) = sum_v v
(softmax sums to 1 over k).  The whole module therefore collapses to

    out[b, q, :] = (sum_c context[b, c, :]) @ Wkv[:, D:] @ Wout

independent of q, query, Wq and mask.  The device kernel computes the
context reduction and the (folded) weight matmul, then broadcasts the
row across the q dimension and writes the full output shard.

Sharding: core c handles batch b = c//2 and output rows
[(c%2)*1024, (c%2+1)*1024).  Each core reads the full context of its
batch (needed for the complete reduction), so context is read twice
across the 8 cores.

Pipeline: the context reduction runs on the PE as an accumulating
ones-vector fp32r matmul chain (csum[1,512] in PSUM), consuming each
1MB DMA unit as it lands (the DVE add chain it replaced lagged the DMA
stream by ~4.5us).  The row is transposed to partition layout with
four k=1 bf16 matmuls, multiplied against bf16-folded weights (single
pass; tolerance is 2e-2), broadcast across PSUM rows by a
column-broadcast stationary operand, and written out as one 16KB-
descriptor DMA on the scalar ring (active queues monopolize the SDMA
engines, so output splits serialize anyway; the scalar engine issues
its DMA directly behind its own copy with zero cross-engine hops).
"""

import numpy as np
import ml_dtypes

from concourse import bacc
import concourse.mybir as mybir
from concourse.tile import TileContext
from concourse.bass_utils import run_bass_kernel_spmd

B, QL, CL, D, H = 4, 2048, 2048, 512, 8
N_CORES = 8
ROWS_PER_CORE = QL // 2  # 1024

F32 = mybir.dt.float32
F32R = mybir.dt.float32r
BF16 = mybir.dt.bfloat16

_NC_CACHE = {}


def _build_nc():
    nc = bacc.Bacc("TRN2", target_bir_lowering=False, enable_partition_id=False,
                   monotonic_sem_count=0)

    ctx_h = nc.dram_tensor("ctx", [CL, D], F32R, kind="ExternalInput")
    # host passes W2 = Wv @ Wout in SBUF layout: [p, c*512+n] = W2[c*128+p, n]
    w2_h = nc.dram_tensor("w2", [128, 4 * D], BF16, kind="ExternalInput")
    out_h = nc.dram_tensor("out", [ROWS_PER_CORE, D], F32, kind="ExternalOutput")

    P = 128
    G = 4            # context DMA units (1 MB each)
    NT = 4           # consecutive rows per partition -> 8KB descriptors
                     # (4KB descriptors measured ~215GB/s vs ~420 at 8KB)
    DC = D // P      # 4 column chunks of 128

    # first 3 units are 1MB (8KB desc, fast regime); the tail tapers
    # 0.5/0.25/0.25MB so the final reduction matmuls start sooner after
    # the stream ends (the whole-tile completion sem gates each group)
    ctx_big = ctx_h[0:1536, :].rearrange("(g p n) d -> g p (n d)", g=3, p=P, n=4)
    ctx_med = ctx_h[1536:1664, :].rearrange("(p n) d -> p (n d)", p=P, n=1)
    ctx_me2 = ctx_h[1664:1792, :].rearrange("(p n) d -> p (n d)", p=P, n=1)
    ctx_sm1 = ctx_h[1792:1920, :].rearrange("(p n) d -> p (n d)", p=P, n=1)
    ctx_sm2 = ctx_h[1920:2048, :].rearrange("(p n) d -> p (n d)", p=P, n=1)

    with TileContext(nc) as tc:
        with (
            tc.tile_pool(name="ctxp", bufs=G) as ctxp,
            tc.tile_pool(name="work", bufs=1) as work,
            tc.tile_pool(name="psum", bufs=1, space="PSUM") as psum,
        ):
            # context first on the sync HWDGE ring; weights queue behind
            tiles = []  # (tile, n_chunks)
            for g in range(3):
                t = ctxp.tile([P, 4 * D], F32R, tag="ctx")
                nc.sync.dma_start(out=t[:], in_=ctx_big[g])
                tiles.append((t, 4))
            tm = ctxp.tile([P, 2 * D], F32R, tag="ctxm")
            nc.sync.dma_start(out=tm[:, 0:D], in_=ctx_med)
            nc.sync.dma_start(out=tm[:, D : 2 * D], in_=ctx_me2)
            ts1 = ctxp.tile([P, D], F32R, tag="ctxs1")
            nc.sync.dma_start(out=ts1[:], in_=ctx_sm1)
            ts2 = ctxp.tile([P, D], F32R, tag="ctxs2")
            nc.sync.dma_start(out=ts2[:], in_=ctx_sm2)
            tiles += [(tm, 2), (ts1, 1), (ts2, 1)]
            w2_sb = work.tile([P, DC * D], BF16, tag="w2_sb")
            nc.sync.dma_start(out=w2_sb[:], in_=w2_h[:, :])

            # constants (memset can't write f32r; copy-cast from f32).
            # All on gpsimd/scalar: with zero DVE instructions the
            # preamble's ~3.7us qDveTable load should disappear.
            ones1f = work.tile([P, 1], F32, tag="ones1f")
            nc.gpsimd.memset(ones1f[:], 1.0)
            ones1 = work.tile([P, 1], F32R, tag="ones1")
            nc.gpsimd.tensor_copy(out=ones1[:], in_=ones1f[:])
            onepf = work.tile([1, 1], F32, tag="onepf")
            nc.gpsimd.memset(onepf[:], 1.0)
            onep = work.tile([1, 1], BF16, tag="onep")
            nc.gpsimd.tensor_copy(out=onep[:], in_=onepf[:])

            # csum[0, d] = sum_rows ctx[row, d]: accumulating PE matmul
            # chain, ones [128,1] stationary, each 512-col chunk streamed
            # as it lands
            csum_ps = psum.tile([1, D], F32, tag="csum_ps")
            n_mm = sum(n for _, n in tiles)
            i = 0
            for t, n_chunks in tiles:
                for k in range(n_chunks):
                    nc.tensor.matmul(
                        csum_ps[:],
                        ones1[:],
                        t[:, k * D : (k + 1) * D],
                        start=(i == 0),
                        stop=(i == n_mm - 1),
                    )
                    i += 1

            # single scalar cast: the DVE half of the old split started
            # 0.3-1.0us late on its sem (PE-stop prop to DVE is slow) and
            # DVE serves no other purpose in this kernel
            csum_sb = work.tile([1, D], BF16, tag="csum_sb")
            nc.scalar.copy(out=csum_sb[:], in_=csum_ps[:])

            # transpose to partition layout: csumT[m, c] = csum[0, c*128+m]
            # via four k=1 rank-1 bf16 matmuls (lhsT = csum slice [1, 128])
            csumT_ps = psum.tile([P, DC], F32, tag="csumT_ps")
            for c in range(DC):
                nc.tensor.matmul(
                    csumT_ps[:, c : c + 1],
                    csum_sb[:, c * P : (c + 1) * P],
                    onep[:],
                    start=True,
                    stop=True,
                )
            csT_bf = work.tile([P, DC], BF16, tag="csT_bf")
            nc.scalar.copy(out=csT_bf[:], in_=csumT_ps[:])

            # o-matmuls with a column-broadcast stationary operand:
            # lhsT[k, m] = csumT[k, c] for every m, so every output row of
            # the (128, 512) PSUM tile is o[n] — the q-broadcast falls out
            # of the matmul for free.  Single bf16 pass (~3e-3 rel err).
            bc_ps = psum.tile([P, D], F32, tag="bc_ps")
            for c in range(DC):
                nc.tensor.matmul(
                    bc_ps[:],
                    csT_bf[:, c : c + 1].broadcast_to([P, P]),
                    w2_sb[:, c * D : (c + 1) * D],
                    start=(c == 0),
                    stop=(c == DC - 1),
                )

            # active queues monopolize the SDMA engines (measured: multi-
            # queue outputs run SEQUENTIALLY, ~31us makespan regardless of
            # split) — so write everything from ONE queue with the
            # earliest possible start: the scalar engine issues its DMA
            # immediately behind its own copy, zero cross-engine sem hops.
            # one copy materializes the row TWICE via a step-0 repeated
            # PSUM source (4KB source runs lift the queue ~385->~420GB/s)
            bcast = work.tile([P, 2 * D], F32, tag="bcast")
            ps = bc_ps[:]
            ps_rep = type(ps)(ps.tensor, ps.offset, [ps.ap[0], [0, 2], ps.ap[1]])
            nc.scalar.copy(out=bcast[:], in_=ps_rep)

            a = bcast[:]
            out_a = out_h[:, :].rearrange("(p j) n -> p (j n)", p=P, j=8)
            rep_a = type(a)(a.tensor, a.offset, [a.ap[0], [0, 4], a.ap[1]])
            nc.scalar.dma_start(out=out_a, in_=rep_a)

    nc.compile()
    return nc


def kernel(query=None, context=None, mask=None, Wq=None, Wkv=None, Wout=None,
           trace=False, **_ignored):
    context = np.asarray(context, dtype=np.float32)
    Wkv = np.asarray(Wkv, dtype=np.float32)
    Wout = np.asarray(Wout, dtype=np.float32)

    # fold the V projection and output projection into one matrix
    W2 = (Wkv[:, D:].astype(np.float64) @ Wout.astype(np.float64)).astype(np.float32)
    # pre-layout to SBUF shape: [p, c*512+n] = W2[c*128+p, n]
    W2sb = np.ascontiguousarray(
        W2.reshape(4, 128, D).transpose(1, 0, 2).reshape(128, 4 * D)
    )
    w2bf = W2sb.astype(ml_dtypes.bfloat16)

    if "nc" not in _NC_CACHE:
        _NC_CACHE["nc"] = _build_nc()
    nc = _NC_CACHE["nc"]

    in_maps = []
    for c in range(N_CORES):
        b = c // 2
        in_maps.append({"ctx": np.ascontiguousarray(context[b]), "w2": w2bf})

    res = run_bass_kernel_spmd(nc, in_maps, core_ids=list(range(N_CORES)),
                               trace=trace)
    kernel.last_results = res

    out = np.empty((B, QL, D), dtype=np.float32)
    for c in range(N_CORES):
        b, h = c // 2, c % 2
        out[b, h * ROWS_PER_CORE : (h + 1) * ROWS_PER_CORE, :] = res.results[c]["out"]
    return out


kernel.last_results = None

